# revision 23
# baseline (speedup 1.0000x reference)
"""ColBERT MaxSim retrieval kernel for 8 Trainium2 NeuronCores.

Problem (per reference):
  Q  = l2norm(q_hidden @ W + b)                    [B, 32, 128]
  PD = l2norm((pd_hidden @ W + b) * pd_mask)       [B, 512, 128]
  ND = l2norm((nd_hidden @ W + b) * nd_mask)       [B, 512, 128]
  pos = einsum(Q, PD).max(k).sum(q);  neg likewise; out = [B, 2]

Sharding: pure data parallelism — batch dim (128) split across 8 cores
(16 batches each); W, b replicated.

Math trick: never materialize normalized PD. With
  S_raw[q,k] = (Qn @ (Xd W + b)^T)[q,k]
  cs[k] = rsqrt(ss[k] + big*(1-mask[k]));  ss[k] = ||Xd_k W + b||^2
the reference score matrix is S_raw * cs (masked columns get cs ~ 1e-11,
so exactly-zero reference columns become ~1e-9 noise), so
pos = sum_q max_k (S_raw * cs).  The doc path is scale-invariant in
(W, b) jointly, so doc projections run on fp8 e4m3 inputs with W
pre-scaled by 64 on the host (keeps 0.02-scale weights out of the fp8
subnormal range); the query path stays bf16 with unscaled W.

Layout / schedule (per core, 16 batches in 4 groups x {pd, nd}):
  - Hidden inputs pre-transposed + cast on the HOST; X^T packed
    [128 part, batch, k-chunk, token]; each group loads as 2x 768 KB
    DMAs split across both HWDGE rings (sync + scalar).
  - fp8 DoubleRow projections (K=256/pass, 3 passes), weight-major
    over 4 batches, into two [128,2,512] 2-bank PSUM tiles.
  - ptb (DVE, +bias) and sq (scalar, Square+bias) both read the PSUM
    projection directly — parallel, 2 batches per instruction.
  - Norms: ones[128,32] stationary replicates each batch's ss across
    its 32 score rows (col-tiled, one PSUM bank); masks fold in via
    ONE K=4 matmul (blkmap x mask rows); one full-tile rsqrt ACT
    yields the score-shaped multiplier.
  - MaxSim: 4 col-tiled matmuls (Qn^T slices stationary) into one
    bank; scr=s4*csr then free-dim max-reduce on DVE.
"""

import os
import sys

import numpy as np

for _p in ("/opt/trn_rl_repo",):
    if _p not in sys.path and os.path.isdir(_p):
        sys.path.insert(0, _p)

import ml_dtypes  # noqa: E402

import concourse.bass as bass  # noqa: E402
import concourse.bacc as bacc  # noqa: E402
import concourse.tile as tile  # noqa: E402
from concourse import mybir  # noqa: E402
from concourse.bass_utils import run_bass_kernel_spmd  # noqa: E402

# Problem shape (hardcoded per contract)
B, LQ, LD, H, D = 128, 32, 512, 768, 128
NCORES = 8
BC = B // NCORES          # 16 batches per core
KT = H // 128             # 6 contraction tiles
W_SCALE = 64.0            # doc-path W,b pre-scale (fp8 range)
MASK_BIG = 1.0e18 * W_SCALE * W_SCALE

F32 = mybir.dt.float32
BF16 = mybir.dt.bfloat16
FP8 = mybir.dt.float8e4
AF = mybir.ActivationFunctionType
ALU = mybir.AluOpType
DR = mybir.MatmulPerfMode.DoubleRow


def build_kernel():
    nc = bacc.Bacc()

    qt_d = nc.dram_tensor("qt", [128, KT * 512], FP8, kind="ExternalInput")
    pdt_d = nc.dram_tensor("pdt", [128, BC * KT * 512], FP8, kind="ExternalInput")
    ndt_d = nc.dram_tensor("ndt", [128, BC * KT * 512], FP8, kind="ExternalInput")
    cb_d = nc.dram_tensor("cb", [128, 4], BF16, kind="ExternalInput")
    bb_d = nc.dram_tensor("bb", [128, 1], F32, kind="ExternalInput")
    w8_d = nc.dram_tensor("W8", [128, KT * 128], FP8, kind="ExternalInput")
    # packed masks (rows 0-3): cols 0:2048 mbp | 2048:4096 mbn | 4096:4224 bm
    cm_d = nc.dram_tensor("cm", [4, 8 * LD + 128], BF16, kind="ExternalInput")
    out_d = nc.dram_tensor("out", [BC, 2], F32, kind="ExternalOutput")

    with tile.TileContext(nc) as tc:
        with (
            tc.tile_pool(name="const", bufs=1) as const,
            tc.tile_pool(name="xin", bufs=3) as xin,
            tc.tile_pool(name="ptb", bufs=4) as ptbp,
            tc.tile_pool(name="sq", bufs=4) as sqp,
            tc.tile_pool(name="small", bufs=4) as smallp,
            tc.tile_pool(name="csr", bufs=2) as csrp,
            tc.tile_pool(name="persist", bufs=1) as persist,
            tc.tile_pool(name="ptps", bufs=2, space="PSUM") as ptpsp,
            tc.tile_pool(name="ssps", bufs=2, space="PSUM") as sspsp,
            tc.tile_pool(name="s4ps", bufs=1, space="PSUM") as s4psp,
            tc.tile_pool(name="bcps", bufs=1, space="PSUM") as bcpsp,
        ):
            # ---- constants: packed DMAs on the sync (HWDGE) ring; the
            # gpsimd SWDGE ring starts on doc data immediately ----
            cb_sb = const.tile([128, 4], BF16)
            nc.sync.dma_start(out=cb_sb, in_=cb_d[:, :])
            bb_sb = const.tile([128, 1], F32)
            nc.sync.dma_start(out=bb_sb, in_=bb_d[:, :])
            w8_sb = const.tile([128, KT, 128], FP8)
            nc.sync.dma_start(
                out=w8_sb, in_=w8_d[:, :].rearrange("p (k d) -> p k d", k=KT)
            )
            cm_sb = const.tile([4, 8 * LD + 128], BF16)
            nc.sync.dma_start(out=cm_sb, in_=cm_d[:, :])

            e4 = cb_sb[:, 0:4]
            b64_sb = bb_sb[:, 0:1]
            bm_sb = cm_sb[:, 8 * LD : 8 * LD + 128]

            ones_col = const.tile([128, 1], BF16)
            nc.vector.memset(ones_col, 1.0)
            ones_row = const.tile([1, 128], BF16)
            nc.vector.memset(ones_row, 1.0)
            ones32 = const.tile([128, 32], BF16)
            nc.vector.memset(ones32, 1.0)

            rm_sb = persist.tile([128, 8], BF16)
            qtn_sb = persist.tile([128, BC * LQ], BF16)

            # ---- HAM warmup: one activity window of dependency-free
            # matmuls so the PE clock ramps while the first DMAs stream ----
            warm_ps = bcpsp.tile([32, 512], F32, tag="bc")
            dummy_sb = const.tile([128, 512], BF16)
            nc.vector.memset(dummy_sb, 0.0)
            for i in range(6):
                nc.tensor.matmul(
                    warm_ps, ones32, dummy_sb, start=(i == 0), stop=(i == 5)
                )

            # ---- doc loop: 8 groups of 4 batches, software-pipelined ----
            # Iteration gi runs group gi's DMA+projections+(ptb,sq) and
            # group gi-1's norms/maxsim/reduce, so the strict-FIFO PE queue
            # never waits on the scalar/DVE chain of the same group.
            groups = [(u, ti) for u in range(4) for ti in range(2)]

            def stage_front(gi):
                u, ti = groups[gi]
                xdram = pdt_d if ti == 0 else ndt_d
                span = xdram[
                    :, u * 4 * KT * 512 : (u + 1) * 4 * KT * 512
                ].rearrange("p (j k l) -> p j k l", j=4, k=KT)
                if gi < 2:
                    # ramp: split across both rings for minimum latency
                    xt4 = xin.tile([128, 4, KT, 512], FP8, tag="xin")
                    nc.gpsimd.dma_start(out=xt4[:, 0:2], in_=span[:, 0:2])
                    nc.sync.dma_start(out=xt4[:, 2:4], in_=span[:, 2:4])
                else:
                    # steady state: one 1.5 MB DMA, rings alternate by group
                    xt4 = xin.tile([128, 4, KT, 512], FP8, tag="xin")
                    if gi % 2 == 0:
                        nc.sync.dma_start(out=xt4, in_=span)
                    else:
                        nc.gpsimd.dma_start(out=xt4, in_=span)
                pt2s = []
                for _h in range(2):
                    pt2 = ptpsp.tile([128, 2, 512], F32, tag="pt")
                    pt2s.append(pt2)
                for i in range(KT // 2):
                    for g in range(4):
                        nc.tensor.matmul(
                            pt2s[g // 2][:, g % 2, :],
                            w8_sb[:, 2 * i : 2 * i + 2, :],
                            xt4[:, g, 2 * i : 2 * i + 2, :],
                            start=(i == 0),
                            stop=(i == KT // 2 - 1),
                            perf_mode=DR,
                        )
                ptbs = []
                sqs = []
                if gi == len(groups) - 1:
                    # fine-grained per-batch ops: shortens the pipeline drain
                    for h in range(2):
                        for jj in range(2):
                            ptb1 = ptbp.tile([128, 512], BF16, tag="ptb")
                            nc.vector.tensor_scalar_add(
                                ptb1, pt2s[h][:, jj, :], b64_sb
                            )
                            sq1 = sqp.tile([128, 512], BF16, tag="sq")
                            nc.scalar.activation(
                                sq1, pt2s[h][:, jj, :], AF.Square, bias=b64_sb
                            )
                            ptbs.append(ptb1)
                            sqs.append(sq1)
                else:
                    for h in range(2):
                        ptb2 = ptbp.tile([128, 2, 512], BF16, tag="ptb")
                        nc.vector.tensor_scalar_add(ptb2, pt2s[h], b64_sb)
                        sq2 = sqp.tile([128, 2, 512], BF16, tag="sq")
                        nc.scalar.activation(sq2, pt2s[h], AF.Square, bias=b64_sb)
                        for jj in range(2):
                            ptbs.append(ptb2[:, jj, :])
                            sqs.append(sq2[:, jj, :])
                return ptbs, sqs

            def stage_back(gi, ptbs, sqs):
                u, ti = groups[gi]
                mb_row = cm_sb[:, ti * 4 * LD + u * LD : ti * 4 * LD + (u + 1) * LD]
                ss4 = sspsp.tile([128, 512], F32, tag="ss")
                for g in range(4):
                    nc.tensor.matmul(
                        ss4[32 * g : 32 * (g + 1), :],
                        ones32,
                        sqs[g],
                        start=True,
                        stop=False,
                        tile_position=(0, 32 * g),
                        skip_group_check=True,
                    )
                nc.tensor.matmul(
                    ss4,
                    bm_sb,
                    mb_row,
                    start=False,
                    stop=True,
                    skip_group_check=True,
                )
                csr = csrp.tile([128, 512], BF16, tag="csr")
                nc.scalar.activation(csr, ss4, AF.Abs_reciprocal_sqrt)
                s4 = s4psp.tile([128, 512], F32, tag="s4")
                for g in range(4):
                    b = 4 * u + g
                    nc.tensor.matmul(
                        s4[32 * g : 32 * (g + 1), :],
                        qtn_sb[:, b * LQ : (b + 1) * LQ],
                        ptbs[g],
                        start=True,
                        stop=True,
                        tile_position=(0, 32 * g),
                    )
                scr = sqp.tile([128, 512], BF16, tag="scr")
                nc.vector.tensor_mul(scr, s4, csr)
                nc.vector.tensor_reduce(
                    rm_sb[:, 2 * u + ti : 2 * u + ti + 1],
                    scr,
                    axis=mybir.AxisListType.X,
                    op=ALU.max,
                )

            prev = stage_front(0)

            # ---- query stage (fp8, scale-invariant under l2norm);
            # qxt DMA queued on sync after group 0 ----
            qxt_sb = const.tile([128, KT, 512], FP8)
            nc.gpsimd.dma_start(
                out=qxt_sb, in_=qt_d[:, :].rearrange("p (k l) -> p k l", k=KT)
            )
            qpt_ps = sspsp.tile([128, 512], F32, tag="ss")
            for i in range(KT // 2):
                nc.tensor.matmul(
                    qpt_ps,
                    w8_sb[:, 2 * i : 2 * i + 2, :],
                    qxt_sb[:, 2 * i : 2 * i + 2, :],
                    start=(i == 0),
                    stop=(i == KT // 2 - 1),
                    perf_mode=DR,
                )
            qtb_sb = ptbp.tile([128, 512], BF16, tag="qtb")
            nc.vector.tensor_scalar_add(qtb_sb, qpt_ps, b64_sb)
            qsq_sb = sqp.tile([128, 512], BF16, tag="qsq")
            nc.vector.tensor_mul(qsq_sb, qtb_sb, qtb_sb)
            qss_ps = sspsp.tile([1, 512], F32, tag="ss")
            nc.tensor.matmul(qss_ps, ones_col, qsq_sb, start=True, stop=True)
            qinv_sb = smallp.tile([1, 512], BF16, tag="inv")
            nc.scalar.activation(qinv_sb, qss_ps, AF.Abs_reciprocal_sqrt)
            qbc_ps = bcpsp.tile([128, 512], F32, tag="bc")
            nc.tensor.matmul(qbc_ps, ones_row, qinv_sb, start=True, stop=True)
            nc.vector.tensor_mul(qtn_sb, qtb_sb, qbc_ps)

            for gi in range(1, len(groups)):
                cur = stage_front(gi)
                stage_back(gi - 1, *prev)
                prev = cur
            stage_back(len(groups) - 1, *prev)

            # ---- final reduction over queries + output ----
            o44_ps = bcpsp.tile([4, 8], F32, tag="bc")
            nc.tensor.matmul(o44_ps, e4, rm_sb, start=True, stop=True)
            o44_sb = smallp.tile([4, 8], F32, tag="o44sb")
            nc.scalar.copy(o44_sb, o44_ps)
            nc.sync.dma_start(
                out=out_d[:, :].rearrange("(u g) t -> g u t", g=4),
                in_=o44_sb.rearrange("g (u t) -> g u t", t=2),
            )

    nc.compile()
    return nc


_NC_CACHE = None


def _get_nc():
    global _NC_CACHE
    if _NC_CACHE is None:
        _NC_CACHE = build_kernel()
    return _NC_CACHE


def _transpose_pack(x, nb, dtype):
    """[nb, L, H] -> [128, nb*KT*512] with free index (b, k, l)."""
    # (p, b, k, l) = x[b, l, 128k+p]
    a = np.asarray(x, dtype=np.float32).astype(dtype)
    a = a.transpose(2, 0, 1)                  # [H, nb, L]
    a = a.reshape(KT, 128, nb, -1)            # [k, p, b, l]
    a = a.transpose(1, 2, 0, 3)               # [p, b, k, l]
    return np.ascontiguousarray(a.reshape(128, -1))


def _mask_pack(mask_core):
    """[16, 512] float mask -> [4, 4*512] f32 big*(1-m), row g col (u,l)."""
    mb = (1.0 - mask_core) * MASK_BIG          # [16, 512]
    return mb.reshape(4, 4, LD).transpose(1, 0, 2).reshape(4, 4 * LD)


def _in_maps(inputs):
    bf16 = ml_dtypes.bfloat16
    fp8 = ml_dtypes.float8_e4m3
    W = np.asarray(inputs["W"], dtype=np.float32)
    w8 = np.ascontiguousarray(
        (W * W_SCALE).reshape(KT, 128, D).transpose(1, 0, 2).reshape(128, KT * D)
    ).astype(fp8)
    b = np.asarray(inputs["b"], dtype=np.float32).reshape(D, 1)
    pm = np.asarray(inputs["pd_mask"], dtype=np.float32)
    nm = np.asarray(inputs["nd_mask"], dtype=np.float32)
    e4 = np.zeros((128, 4), dtype=np.float32)
    for g in range(4):
        e4[32 * g : 32 * (g + 1), g] = 1
    cb = np.ascontiguousarray(e4.astype(bf16))
    bb = np.ascontiguousarray(b * W_SCALE)
    bm = np.zeros((4, 128), dtype=np.float32)
    for g in range(4):
        bm[g, 32 * g : 32 * (g + 1)] = 1
    q = np.asarray(inputs["q_hidden"], dtype=np.float32)
    pd = np.asarray(inputs["pd_hidden"], dtype=np.float32)
    nd = np.asarray(inputs["nd_hidden"], dtype=np.float32)
    maps = []
    for c in range(NCORES):
        sl = slice(c * BC, (c + 1) * BC)
        cm = np.concatenate(
            [_mask_pack(pm[sl]), _mask_pack(nm[sl]), bm], axis=1
        ).astype(bf16)
        maps.append(
            {
                "qt": _transpose_pack(q[sl].reshape(1, BC * LQ, H), 1, fp8),
                "pdt": _transpose_pack(pd[sl], BC, fp8),
                "ndt": _transpose_pack(nd[sl], BC, fp8),
                "cb": cb,
                "bb": bb,
                "W8": w8,
                "cm": np.ascontiguousarray(cm),
            }
        )
    return maps


def run(inputs, **kw):
    """Run on 8 cores; returns (out [128,2] fp32, BassKernelResults)."""
    nc = _get_nc()
    res = run_bass_kernel_spmd(nc, _in_maps(inputs), list(range(NCORES)), **kw)
    out = np.concatenate(
        [np.asarray(res.results[c]["out"], dtype=np.float32) for c in range(NCORES)],
        axis=0,
    )
    return out, res


def kernel(**inputs) -> np.ndarray:
    out, _ = run(inputs)
    return out


# revision 25
# speedup vs baseline: 1.0516x; 1.0516x over previous
"""ColBERT MaxSim retrieval kernel for 8 Trainium2 NeuronCores.

Problem (per reference):
  Q  = l2norm(q_hidden @ W + b)                    [B, 32, 128]
  PD = l2norm((pd_hidden @ W + b) * pd_mask)       [B, 512, 128]
  ND = l2norm((nd_hidden @ W + b) * nd_mask)       [B, 512, 128]
  pos = einsum(Q, PD).max(k).sum(q);  neg likewise; out = [B, 2]

Sharding: pure data parallelism — batch dim (128) split across 8 cores
(16 batches each); W, b replicated.

Math trick: never materialize normalized PD. With
  S_raw[q,k] = (Qn @ (Xd W + b)^T)[q,k]
  cs[k] = rsqrt(ss[k] + big*(1-mask[k]));  ss[k] = ||Xd_k W + b||^2
the reference score matrix is S_raw * cs (masked columns get cs ~ 1e-11,
so exactly-zero reference columns become ~1e-9 noise), so
pos = sum_q max_k (S_raw * cs).  The doc path is scale-invariant in
(W, b) jointly, so doc projections run on fp8 e4m3 inputs with W
pre-scaled by 64 on the host (keeps 0.02-scale weights out of the fp8
subnormal range); the query path stays bf16 with unscaled W.

Layout / schedule (per core, 16 batches in 4 groups x {pd, nd}):
  - Hidden inputs pre-transposed + cast on the HOST; X^T packed
    [128 part, batch, k-chunk, token]; each group loads as 2x 768 KB
    DMAs split across both HWDGE rings (sync + scalar).
  - fp8 DoubleRow projections (K=256/pass, 3 passes), weight-major
    over 4 batches, into two [128,2,512] 2-bank PSUM tiles.
  - ptb (DVE, +bias) and sq (scalar, Square+bias) both read the PSUM
    projection directly — parallel, 2 batches per instruction.
  - Norms: ones[128,32] stationary replicates each batch's ss across
    its 32 score rows (col-tiled, one PSUM bank); masks fold in via
    ONE K=4 matmul (blkmap x mask rows); one full-tile rsqrt ACT
    yields the score-shaped multiplier.
  - MaxSim: 4 col-tiled matmuls (Qn^T slices stationary) into one
    bank; scr=s4*csr then free-dim max-reduce on DVE.
"""

import os
import sys

import numpy as np

for _p in ("/opt/trn_rl_repo",):
    if _p not in sys.path and os.path.isdir(_p):
        sys.path.insert(0, _p)

import ml_dtypes  # noqa: E402

import concourse.bass as bass  # noqa: E402
import concourse.bacc as bacc  # noqa: E402
import concourse.tile as tile  # noqa: E402
from concourse import mybir  # noqa: E402
from concourse.bass_utils import run_bass_kernel_spmd  # noqa: E402

# Problem shape (hardcoded per contract)
B, LQ, LD, H, D = 128, 32, 512, 768, 128
NCORES = 8
BC = B // NCORES          # 16 batches per core
KT = H // 128             # 6 contraction tiles
W_SCALE = 64.0            # doc-path W,b pre-scale (fp8 range)
MASK_BIG = 1.0e18 * W_SCALE * W_SCALE

F32 = mybir.dt.float32
BF16 = mybir.dt.bfloat16
FP8 = mybir.dt.float8e4
AF = mybir.ActivationFunctionType
ALU = mybir.AluOpType
DR = mybir.MatmulPerfMode.DoubleRow


def build_kernel():
    nc = bacc.Bacc()

    qt_d = nc.dram_tensor("qt", [128, KT * 512], FP8, kind="ExternalInput")
    pdt_d = nc.dram_tensor("pdt", [128, BC * KT * 512], FP8, kind="ExternalInput")
    ndt_d = nc.dram_tensor("ndt", [128, BC * KT * 512], FP8, kind="ExternalInput")
    cb_d = nc.dram_tensor("cb", [128, 4], BF16, kind="ExternalInput")
    bb_d = nc.dram_tensor("bb", [128, 1], F32, kind="ExternalInput")
    w8_d = nc.dram_tensor("W8", [128, KT * 128], FP8, kind="ExternalInput")
    # packed masks (rows 0-3): cols 0:2048 mbp | 2048:4096 mbn | 4096:4224 bm
    cm_d = nc.dram_tensor("cm", [4, 8 * LD + 128], BF16, kind="ExternalInput")
    out_d = nc.dram_tensor("out", [BC, 2], F32, kind="ExternalOutput")

    with tile.TileContext(nc) as tc:
        with (
            tc.tile_pool(name="const", bufs=1) as const,
            tc.tile_pool(name="xin", bufs=6) as xin,
            tc.tile_pool(name="ptb", bufs=4) as ptbp,
            tc.tile_pool(name="sq", bufs=4) as sqp,
            tc.tile_pool(name="small", bufs=4) as smallp,
            tc.tile_pool(name="csr", bufs=2) as csrp,
            tc.tile_pool(name="persist", bufs=1) as persist,
            tc.tile_pool(name="ptps", bufs=2, space="PSUM") as ptpsp,
            tc.tile_pool(name="ssps", bufs=2, space="PSUM") as sspsp,
            tc.tile_pool(name="s4ps", bufs=1, space="PSUM") as s4psp,
            tc.tile_pool(name="bcps", bufs=1, space="PSUM") as bcpsp,
        ):
            # ---- constants: packed DMAs on the sync (HWDGE) ring; the
            # gpsimd SWDGE ring starts on doc data immediately ----
            cb_sb = const.tile([128, 4], BF16)
            nc.sync.dma_start(out=cb_sb, in_=cb_d[:, :])
            bb_sb = const.tile([128, 1], F32)
            nc.sync.dma_start(out=bb_sb, in_=bb_d[:, :])
            w8_sb = const.tile([128, KT, 128], FP8)
            nc.sync.dma_start(
                out=w8_sb, in_=w8_d[:, :].rearrange("p (k d) -> p k d", k=KT)
            )
            cm_sb = const.tile([4, 8 * LD + 128], BF16)
            nc.sync.dma_start(out=cm_sb, in_=cm_d[:, :])

            e4 = cb_sb[:, 0:4]
            b64_sb = bb_sb[:, 0:1]
            bm_sb = cm_sb[:, 8 * LD : 8 * LD + 128]

            ones_col = const.tile([128, 1], BF16)
            nc.vector.memset(ones_col, 1.0)
            ones_row = const.tile([1, 128], BF16)
            nc.vector.memset(ones_row, 1.0)
            ones32 = const.tile([128, 32], BF16)
            nc.vector.memset(ones32, 1.0)

            rm_sb = persist.tile([128, 8], BF16)
            qtn_sb = persist.tile([128, BC * LQ], BF16)

            # ---- HAM warmup: one activity window of dependency-free
            # matmuls so the PE clock ramps while the first DMAs stream ----
            warm_ps = bcpsp.tile([32, 512], F32, tag="bc")
            dummy_sb = const.tile([128, 512], BF16)
            nc.vector.memset(dummy_sb, 0.0)
            for i in range(10):
                nc.tensor.matmul(
                    warm_ps, ones32, dummy_sb, start=(i == 0), stop=(i == 9)
                )

            # ---- doc loop: 8 groups of 4 batches, software-pipelined ----
            # Iteration gi runs group gi's DMA+projections+(ptb,sq) and
            # group gi-1's norms/maxsim/reduce, so the strict-FIFO PE queue
            # never waits on the scalar/DVE chain of the same group.
            groups = [(u, ti) for u in range(4) for ti in range(2)]

            def stage_front(gi):
                u, ti = groups[gi]
                xdram = pdt_d if ti == 0 else ndt_d
                halves = []
                for h in range(2):
                    xt2 = xin.tile([128, 2, KT, 512], FP8, tag="xin")
                    src = xdram[
                        :,
                        (u * 4 + 2 * h) * KT * 512 : (u * 4 + 2 * h + 2)
                        * KT
                        * 512,
                    ].rearrange("p (j k l) -> p j k l", j=2, k=KT)
                    if h == 0:
                        nc.gpsimd.dma_start(out=xt2, in_=src)
                    else:
                        nc.sync.dma_start(out=xt2, in_=src)
                    halves.append(xt2)
                pt2s = []
                for _h in range(2):
                    pt2 = ptpsp.tile([128, 2, 512], F32, tag="pt")
                    pt2s.append(pt2)
                for i in range(KT // 2):
                    for g in range(4):
                        nc.tensor.matmul(
                            pt2s[g // 2][:, g % 2, :],
                            w8_sb[:, 2 * i : 2 * i + 2, :],
                            halves[g // 2][:, g % 2, 2 * i : 2 * i + 2, :],
                            start=(i == 0),
                            stop=(i == KT // 2 - 1),
                            perf_mode=DR,
                        )
                ptbs = []
                sqs = []
                if gi == len(groups) - 1:
                    # fine-grained per-batch ops: shortens the pipeline drain
                    for h in range(2):
                        for jj in range(2):
                            ptb1 = ptbp.tile([128, 512], BF16, tag="ptb")
                            nc.vector.tensor_scalar_add(
                                ptb1, pt2s[h][:, jj, :], b64_sb
                            )
                            sq1 = sqp.tile([128, 512], BF16, tag="sq")
                            nc.scalar.activation(
                                sq1, pt2s[h][:, jj, :], AF.Square, bias=b64_sb
                            )
                            ptbs.append(ptb1)
                            sqs.append(sq1)
                else:
                    for h in range(2):
                        ptb2 = ptbp.tile([128, 2, 512], BF16, tag="ptb")
                        nc.vector.tensor_scalar_add(ptb2, pt2s[h], b64_sb)
                        sq2 = sqp.tile([128, 2, 512], BF16, tag="sq")
                        nc.scalar.activation(sq2, pt2s[h], AF.Square, bias=b64_sb)
                        for jj in range(2):
                            ptbs.append(ptb2[:, jj, :])
                            sqs.append(sq2[:, jj, :])
                return ptbs, sqs

            def stage_back(gi, ptbs, sqs):
                u, ti = groups[gi]
                mb_row = cm_sb[:, ti * 4 * LD + u * LD : ti * 4 * LD + (u + 1) * LD]
                ss4 = sspsp.tile([128, 512], F32, tag="ss")
                for g in range(4):
                    nc.tensor.matmul(
                        ss4[32 * g : 32 * (g + 1), :],
                        ones32,
                        sqs[g],
                        start=True,
                        stop=False,
                        tile_position=(0, 32 * g),
                        skip_group_check=True,
                    )
                nc.tensor.matmul(
                    ss4,
                    bm_sb,
                    mb_row,
                    start=False,
                    stop=True,
                    skip_group_check=True,
                )
                csr = csrp.tile([128, 512], BF16, tag="csr")
                nc.scalar.activation(csr, ss4, AF.Abs_reciprocal_sqrt)
                s4 = s4psp.tile([128, 512], F32, tag="s4")
                for g in range(4):
                    b = 4 * u + g
                    nc.tensor.matmul(
                        s4[32 * g : 32 * (g + 1), :],
                        qtn_sb[:, b * LQ : (b + 1) * LQ],
                        ptbs[g],
                        start=True,
                        stop=True,
                        tile_position=(0, 32 * g),
                    )
                scr = sqp.tile([128, 512], BF16, tag="scr")
                nc.vector.tensor_mul(scr, s4, csr)
                nc.vector.tensor_reduce(
                    rm_sb[:, 2 * u + ti : 2 * u + ti + 1],
                    scr,
                    axis=mybir.AxisListType.X,
                    op=ALU.max,
                )

            prev = stage_front(0)

            # ---- query stage (fp8, scale-invariant under l2norm);
            # qxt DMA queued on sync after group 0 ----
            qxt_sb = const.tile([128, KT, 512], FP8)
            nc.gpsimd.dma_start(
                out=qxt_sb, in_=qt_d[:, :].rearrange("p (k l) -> p k l", k=KT)
            )
            qpt_ps = sspsp.tile([128, 512], F32, tag="ss")
            for i in range(KT // 2):
                nc.tensor.matmul(
                    qpt_ps,
                    w8_sb[:, 2 * i : 2 * i + 2, :],
                    qxt_sb[:, 2 * i : 2 * i + 2, :],
                    start=(i == 0),
                    stop=(i == KT // 2 - 1),
                    perf_mode=DR,
                )
            qtb_sb = ptbp.tile([128, 512], BF16, tag="qtb")
            nc.vector.tensor_scalar_add(qtb_sb, qpt_ps, b64_sb)
            qsq_sb = sqp.tile([128, 512], BF16, tag="qsq")
            nc.vector.tensor_mul(qsq_sb, qtb_sb, qtb_sb)
            qss_ps = sspsp.tile([1, 512], F32, tag="ss")
            nc.tensor.matmul(qss_ps, ones_col, qsq_sb, start=True, stop=True)
            qinv_sb = smallp.tile([1, 512], BF16, tag="inv")
            nc.scalar.activation(qinv_sb, qss_ps, AF.Abs_reciprocal_sqrt)
            qbc_ps = bcpsp.tile([128, 512], F32, tag="bc")
            nc.tensor.matmul(qbc_ps, ones_row, qinv_sb, start=True, stop=True)
            nc.vector.tensor_mul(qtn_sb, qtb_sb, qbc_ps)

            for gi in range(1, len(groups)):
                cur = stage_front(gi)
                stage_back(gi - 1, *prev)
                prev = cur
            stage_back(len(groups) - 1, *prev)

            # ---- final reduction over queries + output ----
            o44_ps = bcpsp.tile([4, 8], F32, tag="bc")
            nc.tensor.matmul(o44_ps, e4, rm_sb, start=True, stop=True)
            o44_sb = smallp.tile([4, 8], F32, tag="o44sb")
            nc.scalar.copy(o44_sb, o44_ps)
            nc.sync.dma_start(
                out=out_d[:, :].rearrange("(u g) t -> g u t", g=4),
                in_=o44_sb.rearrange("g (u t) -> g u t", t=2),
            )

    nc.compile()
    return nc


_NC_CACHE = None


def _get_nc():
    global _NC_CACHE
    if _NC_CACHE is None:
        _NC_CACHE = build_kernel()
    return _NC_CACHE


def _transpose_pack(x, nb, dtype):
    """[nb, L, H] -> [128, nb*KT*512] with free index (b, k, l)."""
    # (p, b, k, l) = x[b, l, 128k+p]
    a = np.asarray(x, dtype=np.float32).astype(dtype)
    a = a.transpose(2, 0, 1)                  # [H, nb, L]
    a = a.reshape(KT, 128, nb, -1)            # [k, p, b, l]
    a = a.transpose(1, 2, 0, 3)               # [p, b, k, l]
    return np.ascontiguousarray(a.reshape(128, -1))


def _mask_pack(mask_core):
    """[16, 512] float mask -> [4, 4*512] f32 big*(1-m), row g col (u,l)."""
    mb = (1.0 - mask_core) * MASK_BIG          # [16, 512]
    return mb.reshape(4, 4, LD).transpose(1, 0, 2).reshape(4, 4 * LD)


def _in_maps(inputs):
    bf16 = ml_dtypes.bfloat16
    fp8 = ml_dtypes.float8_e4m3
    W = np.asarray(inputs["W"], dtype=np.float32)
    w8 = np.ascontiguousarray(
        (W * W_SCALE).reshape(KT, 128, D).transpose(1, 0, 2).reshape(128, KT * D)
    ).astype(fp8)
    b = np.asarray(inputs["b"], dtype=np.float32).reshape(D, 1)
    pm = np.asarray(inputs["pd_mask"], dtype=np.float32)
    nm = np.asarray(inputs["nd_mask"], dtype=np.float32)
    e4 = np.zeros((128, 4), dtype=np.float32)
    for g in range(4):
        e4[32 * g : 32 * (g + 1), g] = 1
    cb = np.ascontiguousarray(e4.astype(bf16))
    bb = np.ascontiguousarray(b * W_SCALE)
    bm = np.zeros((4, 128), dtype=np.float32)
    for g in range(4):
        bm[g, 32 * g : 32 * (g + 1)] = 1
    q = np.asarray(inputs["q_hidden"], dtype=np.float32)
    pd = np.asarray(inputs["pd_hidden"], dtype=np.float32)
    nd = np.asarray(inputs["nd_hidden"], dtype=np.float32)
    maps = []
    for c in range(NCORES):
        sl = slice(c * BC, (c + 1) * BC)
        cm = np.concatenate(
            [_mask_pack(pm[sl]), _mask_pack(nm[sl]), bm], axis=1
        ).astype(bf16)
        maps.append(
            {
                "qt": _transpose_pack(q[sl].reshape(1, BC * LQ, H), 1, fp8),
                "pdt": _transpose_pack(pd[sl], BC, fp8),
                "ndt": _transpose_pack(nd[sl], BC, fp8),
                "cb": cb,
                "bb": bb,
                "W8": w8,
                "cm": np.ascontiguousarray(cm),
            }
        )
    return maps


def run(inputs, **kw):
    """Run on 8 cores; returns (out [128,2] fp32, BassKernelResults)."""
    nc = _get_nc()
    res = run_bass_kernel_spmd(nc, _in_maps(inputs), list(range(NCORES)), **kw)
    out = np.concatenate(
        [np.asarray(res.results[c]["out"], dtype=np.float32) for c in range(NCORES)],
        axis=0,
    )
    return out, res


def kernel(**inputs) -> np.ndarray:
    out, _ = run(inputs)
    return out


# revision 26
# speedup vs baseline: 1.0540x; 1.0023x over previous
"""ColBERT MaxSim retrieval kernel for 8 Trainium2 NeuronCores.

Problem (per reference):
  Q  = l2norm(q_hidden @ W + b)                    [B, 32, 128]
  PD = l2norm((pd_hidden @ W + b) * pd_mask)       [B, 512, 128]
  ND = l2norm((nd_hidden @ W + b) * nd_mask)       [B, 512, 128]
  pos = einsum(Q, PD).max(k).sum(q);  neg likewise; out = [B, 2]

Sharding: pure data parallelism — batch dim (128) split across 8 cores
(16 batches each); W, b replicated.

Math trick: never materialize normalized PD. With
  S_raw[q,k] = (Qn @ (Xd W + b)^T)[q,k]
  cs[k] = rsqrt(ss[k] + big*(1-mask[k]));  ss[k] = ||Xd_k W + b||^2
the reference score matrix is S_raw * cs (masked columns get cs ~ 1e-11,
so exactly-zero reference columns become ~1e-9 noise), so
pos = sum_q max_k (S_raw * cs).  The doc path is scale-invariant in
(W, b) jointly, so doc projections run on fp8 e4m3 inputs with W
pre-scaled by 64 on the host (keeps 0.02-scale weights out of the fp8
subnormal range); the query path stays bf16 with unscaled W.

Layout / schedule (per core, 16 batches in 4 groups x {pd, nd}):
  - Hidden inputs pre-transposed + cast on the HOST; X^T packed
    [128 part, batch, k-chunk, token]; each group loads as 2x 768 KB
    DMAs split across both HWDGE rings (sync + scalar).
  - fp8 DoubleRow projections (K=256/pass, 3 passes), weight-major
    over 4 batches, into two [128,2,512] 2-bank PSUM tiles.
  - ptb (DVE, +bias) and sq (scalar, Square+bias) both read the PSUM
    projection directly — parallel, 2 batches per instruction.
  - Norms: ones[128,32] stationary replicates each batch's ss across
    its 32 score rows (col-tiled, one PSUM bank); masks fold in via
    ONE K=4 matmul (blkmap x mask rows); one full-tile rsqrt ACT
    yields the score-shaped multiplier.
  - MaxSim: 4 col-tiled matmuls (Qn^T slices stationary) into one
    bank; scr=s4*csr then free-dim max-reduce on DVE.
"""

import os
import sys

import numpy as np

for _p in ("/opt/trn_rl_repo",):
    if _p not in sys.path and os.path.isdir(_p):
        sys.path.insert(0, _p)

import ml_dtypes  # noqa: E402

import concourse.bass as bass  # noqa: E402
import concourse.bacc as bacc  # noqa: E402
import concourse.tile as tile  # noqa: E402
from concourse import mybir  # noqa: E402
from concourse.bass_utils import run_bass_kernel_spmd  # noqa: E402

# Problem shape (hardcoded per contract)
B, LQ, LD, H, D = 128, 32, 512, 768, 128
NCORES = 8
BC = B // NCORES          # 16 batches per core
KT = H // 128             # 6 contraction tiles
W_SCALE = 64.0            # doc-path W,b pre-scale (fp8 range)
MASK_BIG = 1.0e18 * W_SCALE * W_SCALE

F32 = mybir.dt.float32
BF16 = mybir.dt.bfloat16
FP8 = mybir.dt.float8e4
AF = mybir.ActivationFunctionType
ALU = mybir.AluOpType
DR = mybir.MatmulPerfMode.DoubleRow


def build_kernel():
    nc = bacc.Bacc()

    qt_d = nc.dram_tensor("qt", [128, KT * 512], BF16, kind="ExternalInput")
    pdt_d = nc.dram_tensor("pdt", [128, BC * KT * 512], FP8, kind="ExternalInput")
    ndt_d = nc.dram_tensor("ndt", [128, BC * KT * 512], FP8, kind="ExternalInput")
    # packed bf16 constants: cols 0:768 w16 | 768:772 e4
    cb_d = nc.dram_tensor("cb", [128, KT * 128 + 4], BF16, kind="ExternalInput")
    bb_d = nc.dram_tensor("bb", [128, 2], F32, kind="ExternalInput")
    w8_d = nc.dram_tensor("W8", [128, KT * 128], FP8, kind="ExternalInput")
    # packed masks (rows 0-3): cols 0:2048 mbp | 2048:4096 mbn | 4096:4224 bm
    cm_d = nc.dram_tensor("cm", [4, 8 * LD + 128], BF16, kind="ExternalInput")
    out_d = nc.dram_tensor("out", [BC, 2], F32, kind="ExternalOutput")

    with tile.TileContext(nc) as tc:
        with (
            tc.tile_pool(name="const", bufs=1) as const,
            tc.tile_pool(name="xin", bufs=6) as xin,
            tc.tile_pool(name="ptb", bufs=4) as ptbp,
            tc.tile_pool(name="sq", bufs=4) as sqp,
            tc.tile_pool(name="small", bufs=4) as smallp,
            tc.tile_pool(name="csr", bufs=2) as csrp,
            tc.tile_pool(name="persist", bufs=1) as persist,
            tc.tile_pool(name="ptps", bufs=2, space="PSUM") as ptpsp,
            tc.tile_pool(name="ssps", bufs=2, space="PSUM") as sspsp,
            tc.tile_pool(name="s4ps", bufs=1, space="PSUM") as s4psp,
            tc.tile_pool(name="bcps", bufs=1, space="PSUM") as bcpsp,
        ):
            # ---- constants: packed DMAs on the gpsimd (SWDGE) ring ----
            cb_sb = const.tile([128, KT * 128 + 4], BF16)
            nc.gpsimd.dma_start(out=cb_sb, in_=cb_d[:, :])
            bb_sb = const.tile([128, 2], F32)
            nc.gpsimd.dma_start(out=bb_sb, in_=bb_d[:, :])
            w8_sb = const.tile([128, KT, 128], FP8)
            nc.gpsimd.dma_start(
                out=w8_sb, in_=w8_d[:, :].rearrange("p (k d) -> p k d", k=KT)
            )
            cm_sb = const.tile([4, 8 * LD + 128], BF16)
            nc.gpsimd.dma_start(out=cm_sb, in_=cm_d[:, :])

            w16_sb = cb_sb[:, 0 : KT * 128]
            e4 = cb_sb[:, KT * 128 : KT * 128 + 4]
            bias_sb = bb_sb[:, 0:1]
            b64_sb = bb_sb[:, 1:2]
            bm_sb = cm_sb[:, 8 * LD : 8 * LD + 128]

            ones_col = const.tile([128, 1], BF16)
            nc.vector.memset(ones_col, 1.0)
            ones_row = const.tile([1, 128], BF16)
            nc.vector.memset(ones_row, 1.0)
            ones32 = const.tile([128, 32], BF16)
            nc.vector.memset(ones32, 1.0)

            rm_sb = persist.tile([128, 8], BF16)
            qtn_sb = persist.tile([128, BC * LQ], BF16)

            # ---- HAM warmup: one activity window of dependency-free
            # matmuls so the PE clock ramps while the first DMAs stream ----
            warm_ps = bcpsp.tile([32, 512], F32, tag="bc")
            dummy_sb = const.tile([128, 512], BF16)
            nc.vector.memset(dummy_sb, 0.0)
            for i in range(6):
                nc.tensor.matmul(
                    warm_ps, ones32, dummy_sb, start=(i == 0), stop=(i == 5)
                )

            # ---- doc loop: 8 groups of 4 batches, software-pipelined ----
            # Iteration gi runs group gi's DMA+projections+(ptb,sq) and
            # group gi-1's norms/maxsim/reduce, so the strict-FIFO PE queue
            # never waits on the scalar/DVE chain of the same group.
            groups = [(u, ti) for u in range(4) for ti in range(2)]

            def stage_front(gi):
                u, ti = groups[gi]
                xdram = pdt_d if ti == 0 else ndt_d
                halves = []
                for h in range(2):
                    xt2 = xin.tile([128, 2, KT, 512], FP8, tag="xin")
                    src = xdram[
                        :,
                        (u * 4 + 2 * h) * KT * 512 : (u * 4 + 2 * h + 2)
                        * KT
                        * 512,
                    ].rearrange("p (j k l) -> p j k l", j=2, k=KT)
                    if h == 0:
                        nc.sync.dma_start(out=xt2, in_=src)
                    else:
                        nc.gpsimd.dma_start(out=xt2, in_=src)
                    halves.append(xt2)
                pt2s = []
                for _h in range(2):
                    pt2 = ptpsp.tile([128, 2, 512], F32, tag="pt")
                    pt2s.append(pt2)
                for i in range(KT // 2):
                    for g in range(4):
                        nc.tensor.matmul(
                            pt2s[g // 2][:, g % 2, :],
                            w8_sb[:, 2 * i : 2 * i + 2, :],
                            halves[g // 2][:, g % 2, 2 * i : 2 * i + 2, :],
                            start=(i == 0),
                            stop=(i == KT // 2 - 1),
                            perf_mode=DR,
                        )
                ptbs = []
                sqs = []
                for h in range(2):
                    ptb2 = ptbp.tile([128, 2, 512], BF16, tag="ptb")
                    nc.vector.tensor_scalar_add(ptb2, pt2s[h], b64_sb)
                    sq2 = sqp.tile([128, 2, 512], BF16, tag="sq")
                    nc.scalar.activation(sq2, pt2s[h], AF.Square, bias=b64_sb)
                    for jj in range(2):
                        ptbs.append(ptb2[:, jj, :])
                        sqs.append(sq2[:, jj, :])
                return ptbs, sqs

            def stage_back(gi, ptbs, sqs):
                u, ti = groups[gi]
                mb_row = cm_sb[:, ti * 4 * LD + u * LD : ti * 4 * LD + (u + 1) * LD]
                ss4 = sspsp.tile([128, 512], F32, tag="ss")
                for g in range(4):
                    nc.tensor.matmul(
                        ss4[32 * g : 32 * (g + 1), :],
                        ones32,
                        sqs[g],
                        start=True,
                        stop=False,
                        tile_position=(0, 32 * g),
                        skip_group_check=True,
                    )
                nc.tensor.matmul(
                    ss4,
                    bm_sb,
                    mb_row,
                    start=False,
                    stop=True,
                    skip_group_check=True,
                )
                csr = csrp.tile([128, 512], BF16, tag="csr")
                nc.scalar.activation(csr, ss4, AF.Abs_reciprocal_sqrt)
                s4 = s4psp.tile([128, 512], F32, tag="s4")
                for g in range(4):
                    b = 4 * u + g
                    nc.tensor.matmul(
                        s4[32 * g : 32 * (g + 1), :],
                        qtn_sb[:, b * LQ : (b + 1) * LQ],
                        ptbs[g],
                        start=True,
                        stop=True,
                        tile_position=(0, 32 * g),
                    )
                scr = sqp.tile([128, 512], BF16, tag="scr")
                nc.vector.tensor_mul(scr, s4, csr)
                nc.vector.tensor_reduce(
                    rm_sb[:, 2 * u + ti : 2 * u + ti + 1],
                    scr,
                    axis=mybir.AxisListType.X,
                    op=ALU.max,
                )

            prev = stage_front(0)

            # ---- query stage (bf16); qxt DMA queued on sync after d0h0 ----
            qxt_sb = const.tile([128, KT * 512], BF16)
            nc.sync.dma_start(out=qxt_sb, in_=qt_d[:, :])
            qpt_ps = sspsp.tile([128, 512], F32, tag="ss")
            for k in range(KT):
                nc.tensor.matmul(
                    qpt_ps,
                    w16_sb[:, 128 * k : 128 * (k + 1)],
                    qxt_sb[:, 512 * k : 512 * (k + 1)],
                    start=(k == 0),
                    stop=(k == KT - 1),
                )
            qtb_sb = ptbp.tile([128, 512], BF16, tag="qtb")
            nc.vector.tensor_scalar_add(qtb_sb, qpt_ps, bias_sb)
            qsq_sb = sqp.tile([128, 512], BF16, tag="qsq")
            nc.vector.tensor_mul(qsq_sb, qtb_sb, qtb_sb)
            qss_ps = sspsp.tile([1, 512], F32, tag="ss")
            nc.tensor.matmul(qss_ps, ones_col, qsq_sb, start=True, stop=True)
            qinv_sb = smallp.tile([1, 512], BF16, tag="inv")
            nc.scalar.activation(qinv_sb, qss_ps, AF.Abs_reciprocal_sqrt)
            qbc_ps = bcpsp.tile([128, 512], F32, tag="bc")
            nc.tensor.matmul(qbc_ps, ones_row, qinv_sb, start=True, stop=True)
            nc.vector.tensor_mul(qtn_sb, qtb_sb, qbc_ps)

            for gi in range(1, len(groups)):
                cur = stage_front(gi)
                stage_back(gi - 1, *prev)
                prev = cur
            stage_back(len(groups) - 1, *prev)

            # ---- final reduction over queries + output ----
            o44_ps = bcpsp.tile([4, 8], F32, tag="bc")
            nc.tensor.matmul(o44_ps, e4, rm_sb, start=True, stop=True)
            o44_sb = smallp.tile([4, 8], F32, tag="o44sb")
            nc.scalar.copy(o44_sb, o44_ps)
            nc.sync.dma_start(
                out=out_d[:, :].rearrange("(u g) t -> g u t", g=4),
                in_=o44_sb.rearrange("g (u t) -> g u t", t=2),
            )

    nc.compile()
    return nc


_NC_CACHE = None


def _get_nc():
    global _NC_CACHE
    if _NC_CACHE is None:
        _NC_CACHE = build_kernel()
    return _NC_CACHE


def _transpose_pack(x, nb, dtype):
    """[nb, L, H] -> [128, nb*KT*512] with free index (b, k, l)."""
    # (p, b, k, l) = x[b, l, 128k+p]
    a = np.asarray(x, dtype=np.float32).astype(dtype)
    a = a.transpose(2, 0, 1)                  # [H, nb, L]
    a = a.reshape(KT, 128, nb, -1)            # [k, p, b, l]
    a = a.transpose(1, 2, 0, 3)               # [p, b, k, l]
    return np.ascontiguousarray(a.reshape(128, -1))


def _mask_pack(mask_core):
    """[16, 512] float mask -> [4, 4*512] f32 big*(1-m), row g col (u,l)."""
    mb = (1.0 - mask_core) * MASK_BIG          # [16, 512]
    return mb.reshape(4, 4, LD).transpose(1, 0, 2).reshape(4, 4 * LD)


def _in_maps(inputs):
    bf16 = ml_dtypes.bfloat16
    fp8 = ml_dtypes.float8_e4m3
    W = np.asarray(inputs["W"], dtype=np.float32)
    w16 = W.reshape(KT, 128, D).transpose(1, 0, 2).reshape(128, KT * D)
    w8 = np.ascontiguousarray(
        (W * W_SCALE).reshape(KT, 128, D).transpose(1, 0, 2).reshape(128, KT * D)
    ).astype(fp8)
    b = np.asarray(inputs["b"], dtype=np.float32).reshape(D, 1)
    pm = np.asarray(inputs["pd_mask"], dtype=np.float32)
    nm = np.asarray(inputs["nd_mask"], dtype=np.float32)
    e4 = np.zeros((128, 4), dtype=np.float32)
    for g in range(4):
        e4[32 * g : 32 * (g + 1), g] = 1
    # cb pack: w16 | e4
    cb = np.ascontiguousarray(np.concatenate([w16, e4], axis=1).astype(bf16))
    bb = np.ascontiguousarray(np.concatenate([b, b * W_SCALE], axis=1))
    bm = np.zeros((4, 128), dtype=np.float32)
    for g in range(4):
        bm[g, 32 * g : 32 * (g + 1)] = 1
    q = np.asarray(inputs["q_hidden"], dtype=np.float32)
    pd = np.asarray(inputs["pd_hidden"], dtype=np.float32)
    nd = np.asarray(inputs["nd_hidden"], dtype=np.float32)
    maps = []
    for c in range(NCORES):
        sl = slice(c * BC, (c + 1) * BC)
        cm = np.concatenate(
            [_mask_pack(pm[sl]), _mask_pack(nm[sl]), bm], axis=1
        ).astype(bf16)
        maps.append(
            {
                "qt": _transpose_pack(q[sl].reshape(1, BC * LQ, H), 1, bf16),
                "pdt": _transpose_pack(pd[sl], BC, fp8),
                "ndt": _transpose_pack(nd[sl], BC, fp8),
                "cb": cb,
                "bb": bb,
                "W8": w8,
                "cm": np.ascontiguousarray(cm),
            }
        )
    return maps


def run(inputs, **kw):
    """Run on 8 cores; returns (out [128,2] fp32, BassKernelResults)."""
    nc = _get_nc()
    res = run_bass_kernel_spmd(nc, _in_maps(inputs), list(range(NCORES)), **kw)
    out = np.concatenate(
        [np.asarray(res.results[c]["out"], dtype=np.float32) for c in range(NCORES)],
        axis=0,
    )
    return out, res


def kernel(**inputs) -> np.ndarray:
    out, _ = run(inputs)
    return out
